# revision 6
# baseline (speedup 1.0000x reference)
"""Trainium2 Bass kernel for GCNConv + LeakyReLU + LayerNorm (GNN message passing).

Reference computation (single nn.Module forward):
    ew   = |edge_attr[:, 0]|
    add self-loops (weight 1.0), symmetric degree norm:
      deg[c]  = sum_{e: col_e == c} w_e            (incl. self-loops)
      dinv    = deg > 0 ? 1/sqrt(deg) : 0
      norm_e  = dinv[row_e] * w_e * dinv[col_e]
    h    = x @ W.T + b
    out  = segment_sum(h[row] * norm, col)
    out  = LeakyReLU(out, 0.01); out = LayerNorm(out) * gamma + beta

Device strategy (8 NeuronCores, SPMD single NEFF):
  * Nodes padded to 10240 = 80 chunks of 128. Core k owns chunks
    [10k, 10k+10) (target/"col" sharding). Host routes each edge
    (incl. synthesized self-loop edges) to the core owning its target
    chunk; edges are grouped per chunk and padded to a uniform tile
    count T (w=0 padding slots are mathematically inert).
  * Per-edge normalization is factored:
       out[c] = dinv[c] * sum_e w_e * (dinv[row_e]*h[row_e])
    so only per-node scaling is needed: hp = h * dinv (the "h'" table).
  * Phase A (deg):  per 128-edge tile build S'[e, j] = w_e * (colrel_e == j)
    with ONE VectorE tensor_scalar (is_equal then mult, per-partition
    scalars), then PE matmul deg_chunk += S'^T @ ones. deg is exchanged
    with an 8-core AllGather (40 KB).
  * Phase B: h = x @ W.T + b on PE (fp16 inputs, f32 PSUM), scaled by
    dinv into an HBM table hp[10240, 128] fp16.
  * Phase C (messages): per chunk, dma_gather rows hp[row_e] (256 B rows)
    into SBUF, rebuild S', accumulate PSUM msg += S'^T @ G over the
    chunk's tiles, then dinv-scale + LeakyReLU + LayerNorm and DMA out.
  * Scatter-free: segment-sum is done by the PE matmuls, so there are no
    read-modify-write races anywhere.

Host-side work is limited to sharding/layout: slicing, permuting edges
into chunk groups, padding, dtype casts of index data, and the reassembly
of per-core output slices.
"""

import os

import numpy as np

import concourse.bacc as bacc
import concourse.bass as bass
import concourse.mybir as mybir
import concourse.tile as tile
from concourse import bass_utils
from concourse.masks import make_identity

P = 128
D = 128
N_NODES = 10000
N_EDGES = 640000
N_CORES = 8
CPC = 10  # chunks per core
CHUNKS = N_CORES * CPC  # 80
N_PAD = CHUNKS * P  # 10240
LN_EPS = 1e-5
NEG_SLOPE = 0.01

f32 = mybir.dt.float32
f16 = mybir.dt.float16
i16 = mybir.dt.int16

# Results of the last hardware run (for test harnesses to inspect).
LAST_RESULTS = None


# --------------------------------------------------------------------------
# Device program
# --------------------------------------------------------------------------

def build_program(nc, T, n_cores=N_CORES, cpc=CPC, npad=N_PAD, g_tiles=16,
                  single_packet=False):
    """Emit the SPMD program. T = tiles (of 128 edges) per node chunk.

    Each dma_gather call covers at most g_tiles tiles: the SWDGE descriptor
    ring holds dynamic_dma_scratch_size/64 descriptors per engine and a
    gather needs num_idxs/16 + 1 ring slots, which must also leave room for
    the next call to pipeline."""
    chunks = n_cores * cpc
    AX = mybir.AxisListType
    OP = mybir.AluOpType
    ACT = mybir.ActivationFunctionType
    ST = T * P // 16  # int16 idx columns per chunk

    # ---- I/O tensors -----------------------------------------------------
    x_d = nc.dram_tensor("x_pad", [npad, D], f32, kind="ExternalInput")
    W_d = nc.dram_tensor("W", [D, D], f32, kind="ExternalInput")
    b_d = nc.dram_tensor("b_row", [1, D], f32, kind="ExternalInput")
    gam_d = nc.dram_tensor("gamma_row", [1, D], f32, kind="ExternalInput")
    bet_d = nc.dram_tensor("beta_row", [1, D], f32, kind="ExternalInput")
    colrel_d = nc.dram_tensor("colrel", [P, cpc * T], f32, kind="ExternalInput")
    eap_d = nc.dram_tensor("eap", [P, cpc * T], f32, kind="ExternalInput")
    rows_d = nc.dram_tensor("rows16", [P, cpc * ST], i16, kind="ExternalInput")
    out_d = nc.dram_tensor("out", [cpc * P, D], f32, kind="ExternalOutput")

    with tile.TileContext(nc) as tc:
        with (
            tc.tile_pool(name="const", bufs=1) as cp,
            tc.tile_pool(name="edges", bufs=1) as ep,
            tc.tile_pool(name="sb", bufs=3) as sb,
            tc.tile_pool(name="gbuf", bufs=3) as gp,
            tc.tile_pool(name="psum", bufs=2, space="PSUM") as pp,
            tc.tile_pool(name="pacc", bufs=2, space="PSUM") as pa,
            tc.tile_pool(name="dram", bufs=1, space="DRAM") as dp,
        ):
            # ---- constants & edge metadata ------------------------------
            iota_t = cp.tile([P, P], f32)
            nc.gpsimd.iota(
                iota_t[:], pattern=[[1, P]], base=0, channel_multiplier=0,
                allow_small_or_imprecise_dtypes=True,
            )
            ident = cp.tile([P, P], f32)
            make_identity(nc, ident[:])
            ones16 = cp.tile([P, 1], f16)
            nc.vector.memset(ones16[:], 1.0)

            colrel = ep.tile([P, cpc * T], f32)
            nc.sync.dma_start(colrel[:], colrel_d[:, :])
            eap = ep.tile([P, cpc * T], f32)
            nc.sync.dma_start(eap[:], eap_d[:, :])
            rows16 = ep.tile([P, cpc * ST], i16)
            nc.sync.dma_start(rows16[:], rows_d[:, :])
            w_sb = ep.tile([P, cpc * T], f32)
            nc.scalar.activation(w_sb[:], eap[:], ACT.Abs)

            W_sb = cp.tile([P, D], f32)
            nc.sync.dma_start(W_sb[:], W_d[:, :])
            WT_ps = pp.tile([P, D], f32, tag="hps")
            nc.tensor.transpose(WT_ps[:], W_sb[:], ident[:])
            WT16 = cp.tile([P, D], f16)
            nc.vector.tensor_copy(WT16[:], WT_ps[:])

            # b / gamma / beta broadcast to all partitions
            brow = cp.tile([1, D], f32)
            nc.sync.dma_start(brow[:], b_d[:, :])
            b_t = cp.tile([P, D], f32)
            nc.gpsimd.partition_broadcast(b_t[:], brow[:])
            grow = cp.tile([1, D], f32)
            nc.sync.dma_start(grow[:], gam_d[:, :])
            g_t = cp.tile([P, D], f32)
            nc.gpsimd.partition_broadcast(g_t[:], grow[:])
            trow = cp.tile([1, D], f32)
            nc.sync.dma_start(trow[:], bet_d[:, :])
            be_t = cp.tile([P, D], f32)
            nc.gpsimd.partition_broadcast(be_t[:], trow[:])

            # ---- Phase B1: h = x @ W.T + b  (all chunks, fp16, in SBUF) --
            h_all = cp.tile([P, chunks * D], f16)
            for c in range(chunks):
                xc = sb.tile([P, D], f32, tag="xc")
                nc.sync.dma_start(xc[:], x_d[c * P:(c + 1) * P, :])
                xT_ps = pp.tile([P, D], f32, tag="xT")
                nc.tensor.transpose(xT_ps[:], xc[:], ident[:])
                xT16 = sb.tile([P, D], f16, tag="xT16")
                nc.vector.tensor_copy(xT16[:], xT_ps[:])
                h_ps = pp.tile([P, D], f32, tag="hps")
                nc.tensor.matmul(h_ps[:], lhsT=xT16[:], rhs=WT16[:],
                                 start=True, stop=True)
                nc.vector.tensor_tensor(
                    out=h_all[:, c * D:(c + 1) * D], in0=h_ps[:], in1=b_t[:],
                    op=OP.add,
                )

            # ---- Phase A: weighted degree -------------------------------
            deg_loc = cp.tile([P, cpc], f32)
            for c in range(cpc):
                dps = pa.tile([P, 1], f32, tag="dacc")
                for t in range(T):
                    j = c * T + t
                    sp = sb.tile([P, P], f16, tag="sprime")
                    nc.vector.tensor_scalar(
                        out=sp[:], in0=iota_t[:],
                        scalar1=colrel[:, j:j + 1], scalar2=w_sb[:, j:j + 1],
                        op0=OP.is_equal, op1=OP.mult,
                    )
                    nc.tensor.matmul(dps[:], lhsT=sp[:], rhs=ones16[:],
                                     start=(t == 0), stop=(t == T - 1))
                nc.vector.tensor_copy(deg_loc[:, c:c + 1], dps[:])

            # exchange: AllGather degrees of all cores
            deg_in = dp.tile([P, cpc], f32)
            nc.sync.dma_start(deg_in[:], deg_loc[:])
            deg_all = dp.tile([n_cores * P, cpc], f32)
            nc.gpsimd.collective_compute(
                "AllGather", OP.bypass,
                replica_groups=[list(range(n_cores))],
                ins=[deg_in[:].opt()], outs=[deg_all[:].opt()],
            )
            deg_sb = cp.tile([P, chunks], f32)
            for k in range(n_cores):
                nc.sync.dma_start(
                    deg_sb[:, k * cpc:(k + 1) * cpc],
                    deg_all[k * P:(k + 1) * P, :],
                )

            def make_dinv(deg_ap, n, tag):
                mask = sb.tile([P, n], f32, tag=tag + "m")
                nc.vector.tensor_scalar(out=mask[:], in0=deg_ap, scalar1=0.0,
                                        scalar2=None, op0=OP.is_gt)
                dsafe = sb.tile([P, n], f32, tag=tag + "s")
                nc.vector.tensor_scalar(out=dsafe[:], in0=deg_ap, scalar1=1e-12,
                                        scalar2=None, op0=OP.max)
                rec = sb.tile([P, n], f32, tag=tag + "r")
                nc.vector.reciprocal(rec[:], dsafe[:])
                dsq = sb.tile([P, n], f32, tag=tag + "q")
                nc.scalar.sqrt(dsq[:], rec[:])
                dinv = cp.tile([P, n], f32, tag=tag)
                nc.vector.tensor_tensor(out=dinv[:], in0=dsq[:], in1=mask[:],
                                        op=OP.mult)
                return dinv

            dinv_all = make_dinv(deg_sb[:], chunks, "dinva")
            dinv_loc = make_dinv(deg_loc[:], cpc, "dinvl")

            # ---- Phase B2: hp = h * dinv -> HBM table -------------------
            hp_dram = dp.tile([npad, D], f16)
            for c in range(chunks):
                hp_c = sb.tile([P, D], f16, tag="hpc")
                nc.vector.tensor_scalar(
                    out=hp_c[:], in0=h_all[:, c * D:(c + 1) * D],
                    scalar1=dinv_all[:, c:c + 1], scalar2=None, op0=OP.mult,
                )
                nc.sync.dma_start(hp_dram[c * P:(c + 1) * P, :], hp_c[:])

            # ---- Phase C: gather + segment-matmul + LN ------------------
            inv_d = 1.0 / D
            # split each chunk's gather into ring-sized pieces
            tsp = [g_tiles] * (T // g_tiles)
            if T % g_tiles:
                tsp.append(T % g_tiles)
            for c in range(cpc):
                gts = []
                t0 = 0
                for s, tn in enumerate(tsp):
                    if tn == 0:
                        continue
                    gt = gp.tile([P, tn, D], f16, tag="G")
                    i0 = c * ST + t0 * (P // 16)
                    i1 = i0 + tn * (P // 16)
                    nc.gpsimd.dma_gather(
                        out_ap=gt[:], in_ap=hp_dram[:, :],
                        idxs_ap=rows16[:, i0:i1],
                        num_idxs=tn * P, num_idxs_reg=tn * P,
                        elem_size=D, single_packet=single_packet,
                    )
                    gts.append((t0, tn, gt))
                    t0 += tn

                mps = pa.tile([P, D], f32, tag="macc")
                first = True
                for (t0, tn, gt) in gts:
                    for ti in range(tn):
                        t = t0 + ti
                        j = c * T + t
                        sp = sb.tile([P, P], f16, tag="sprime")
                        nc.vector.tensor_scalar(
                            out=sp[:], in0=iota_t[:],
                            scalar1=colrel[:, j:j + 1],
                            scalar2=w_sb[:, j:j + 1],
                            op0=OP.is_equal, op1=OP.mult,
                        )
                        nc.tensor.matmul(mps[:], lhsT=sp[:], rhs=gt[:, ti, :],
                                         start=first, stop=(t == T - 1))
                        first = False

                # tail: dinv scale, LeakyReLU, LayerNorm
                o1 = sb.tile([P, D], f32, tag="o1")
                nc.vector.tensor_scalar(
                    out=o1[:], in0=mps[:], scalar1=dinv_loc[:, c:c + 1],
                    scalar2=None, op0=OP.mult,
                )
                o2 = sb.tile([P, D], f32, tag="o2")
                nc.vector.scalar_tensor_tensor(
                    out=o2[:], in0=o1[:], scalar=NEG_SLOPE, in1=o1[:],
                    op0=OP.mult, op1=OP.max,
                )
                s1 = sb.tile([P, 1], f32, tag="s1")
                nc.vector.reduce_sum(s1[:], o2[:], axis=AX.X)
                nm = sb.tile([P, 1], f32, tag="nm")
                nc.vector.tensor_scalar(out=nm[:], in0=s1[:], scalar1=-inv_d,
                                        scalar2=None, op0=OP.mult)
                cen = sb.tile([P, D], f32, tag="cen")
                nc.vector.tensor_scalar(out=cen[:], in0=o2[:],
                                        scalar1=nm[:, 0:1], scalar2=None,
                                        op0=OP.add)
                sq = sb.tile([P, D], f32, tag="sq")
                ss = sb.tile([P, 1], f32, tag="ss")
                nc.scalar.activation(sq[:], cen[:], ACT.Square, accum_out=ss[:])
                m1 = sb.tile([P, 1], f32, tag="m1")
                nc.vector.tensor_scalar(out=m1[:], in0=ss[:], scalar1=inv_d,
                                        scalar2=LN_EPS, op0=OP.mult, op1=OP.add)
                r1 = sb.tile([P, 1], f32, tag="r1")
                nc.vector.reciprocal(r1[:], m1[:])
                rstd = sb.tile([P, 1], f32, tag="rstd")
                nc.scalar.sqrt(rstd[:], r1[:])
                o3 = sb.tile([P, D], f32, tag="o3")
                nc.vector.scalar_tensor_tensor(
                    out=o3[:], in0=cen[:], scalar=rstd[:, 0:1], in1=g_t[:],
                    op0=OP.mult, op1=OP.mult,
                )
                o4 = sb.tile([P, D], f32, tag="o4")
                nc.vector.tensor_tensor(out=o4[:], in0=o3[:], in1=be_t[:],
                                        op=OP.add)
                nc.sync.dma_start(out_d[c * P:(c + 1) * P, :], o4[:])

    return nc


# --------------------------------------------------------------------------
# Host-side sharding
# --------------------------------------------------------------------------

def shard_inputs(x, edge_attr, W, b, gamma, beta, edge_index,
                 n_cores=N_CORES, cpc=CPC, npad=N_PAD, n_nodes=N_NODES):
    """Route edges (plus synthesized self-loops) to target-chunk groups,
    pad to a uniform per-chunk tile count T, and build per-core input maps.
    Returns (in_maps, T)."""
    chunks = n_cores * cpc
    row = np.asarray(edge_index[0], dtype=np.int64)
    col = np.asarray(edge_index[1], dtype=np.int64)
    ea0 = np.ascontiguousarray(np.asarray(edge_attr)[:, 0], dtype=np.float32)

    loop = np.arange(n_nodes, dtype=np.int64)
    row_all = np.concatenate([row, loop])
    col_all = np.concatenate([col, loop])
    ea_all = np.concatenate([ea0, np.ones(n_nodes, np.float32)])

    chunk_of = (col_all >> 7).astype(np.int64)  # col // 128
    order = np.argsort(chunk_of, kind="stable")
    ch_sorted = chunk_of[order]
    counts = np.bincount(chunk_of, minlength=chunks)
    T = int(np.ceil(counts.max() / P))
    C = T * P

    starts = np.zeros(chunks + 1, np.int64)
    starts[1:] = np.cumsum(counts)
    pos = np.arange(len(order)) - starts[ch_sorted]

    rows_p = np.zeros((chunks, C), np.int16)
    colrel_p = np.zeros((chunks, C), np.float32)
    ea_p = np.zeros((chunks, C), np.float32)
    rows_p[ch_sorted, pos] = row_all[order].astype(np.int16)
    colrel_p[ch_sorted, pos] = (col_all[order] & 127).astype(np.float32)
    ea_p[ch_sorted, pos] = ea_all[order]

    x_pad = np.zeros((npad, D), np.float32)
    x_pad[:n_nodes] = np.asarray(x, dtype=np.float32)
    W_f = np.asarray(W, dtype=np.float32)
    b_r = np.asarray(b, dtype=np.float32).reshape(1, D)
    g_r = np.asarray(gamma, dtype=np.float32).reshape(1, D)
    be_r = np.asarray(beta, dtype=np.float32).reshape(1, D)

    in_maps = []
    for k in range(n_cores):
        sl = slice(k * cpc, (k + 1) * cpc)
        # [chunk, C] -> [P, chunk*T]: edge i of a chunk at (partition i%128,
        # tile i//128), matching the dma_gather / matmul layout.
        cr = colrel_p[sl].reshape(cpc, T, P).transpose(2, 0, 1).reshape(P, cpc * T)
        ea = ea_p[sl].reshape(cpc, T, P).transpose(2, 0, 1).reshape(P, cpc * T)
        # gather idx: position i at (partition i%16, col i//16), tiled x8
        r16 = rows_p[sl].reshape(cpc * T * 8, 16).transpose(1, 0)  # [16, cpc*ST]
        r16 = np.tile(r16, (8, 1))
        in_maps.append({
            "x_pad": x_pad,
            "W": W_f,
            "b_row": b_r,
            "gamma_row": g_r,
            "beta_row": be_r,
            "colrel": np.ascontiguousarray(cr),
            "eap": np.ascontiguousarray(ea),
            "rows16": np.ascontiguousarray(r16),
        })
    return in_maps, T


# --------------------------------------------------------------------------
# Entry point
# --------------------------------------------------------------------------

_prog_cache = {}


def _get_program(T):
    if T not in _prog_cache:
        nc = bacc.Bacc(
            "TRN2",
            target_bir_lowering=False,
            debug=False,
            enable_asserts=False,
            num_devices=N_CORES,
            dynamic_dma_scratch_size=32768,
        )
        build_program(nc, T)
        nc.compile()
        _prog_cache[T] = nc
    return _prog_cache[T]


def kernel(x, edge_attr, W, b, gamma, beta, edge_index):
    global LAST_RESULTS
    in_maps, T = shard_inputs(x, edge_attr, W, b, gamma, beta, edge_index)
    nc = _get_program(T)
    res = bass_utils.run_bass_kernel_spmd(
        nc, in_maps, core_ids=list(range(N_CORES)),
        trace=bool(int(os.environ.get("GNN_TRACE", "0"))),
    )
    LAST_RESULTS = res
    out = np.concatenate([r["out"] for r in res.results], axis=0)
    return out[:N_NODES].astype(np.float32)
